# revision 27
# baseline (speedup 1.0000x reference)
"""Trainium2 Bass kernel for nn_EpisodicMemory (BitNet projections + memory cross-attention).

kernel(**inputs) takes FULL unsharded numpy inputs, returns FULL output
[8, 4096, 1024] f32. Batch-parallel across 8 NeuronCores; two scalar
AllReduce(max) collectives provide the global BitNet activation scales.

Math per core (batch element), mirroring the reference exactly:
  s_x   = max|x| (global) / 127 ;  qx = rne(x/s_x)  (ints, exact in fp16)
  qWk   = sign(Wk) * (|Wk| > 0.5*mean|Wk|)          (ternary, exact in fp16)
  qk    = (qx @ qWk^T) * (mean|Wk| * s_x) + bk
  probs = softmax(qk @ mk^T / sqrt(Dm))             (exp with -8 logit shift)
  h     = x + probs @ mv
  out   = rne(h/s_h) @ qWo^T * (mean|Wo| * s_h) + bo,  s_h = max|h|(global)/127
All matmuls fp16 on PE (BitNet ones exact); softmax/quantization f32.

DMA ring split: nc.sync = loads + xbar transposes, nc.scalar = stores,
nc.gpsimd (SWDGE) = weight/bias loads — avoids FIFO head-of-line blocking.
Phase B/C software-pipeline the next tile's load/quantize/transpose chain.
"""

import math
import time

import numpy as np

import concourse.bass as bass
import concourse.tile as tile
from concourse import bacc, bass_isa, mybir
from concourse.bass_utils import run_bass_kernel_spmd

F32 = mybir.dt.float32
F16 = mybir.dt.float16
F8 = mybir.dt.float8e4

N_CORES = 8
MAGIC = 1.5 * (2.0 ** 23)   # fp32 RNE rounding trick
EXP_SHIFT = 8.0             # logit shift so exp() fits fp16

B, S_FULL, E_DIM, DM_DIM, M_DIM = 8, 4096, 1024, 1024, 2048


def build_nc(S=S_FULL, E=E_DIM, DM=DM_DIM, M=M_DIM, T=512, repeat=1,
             use_collectives=True, with_bias=True):
    assert S % T == 0 and T % 128 == 0 and E % 512 == 0 and DM % 128 == 0 and M % 128 == 0
    NT = S // T
    TS = T // 128
    NE = E // 128
    ND = DM // 128
    NM = M // 128
    NOH = E // 512
    NTC = T // 512

    nc = bacc.Bacc("TRN2", target_bir_lowering=False, debug=False,
                   num_devices=N_CORES)

    x_in = nc.dram_tensor("x", [S, E], F32, kind="ExternalInput").ap()
    mk_in = nc.dram_tensor("memory_keys", [M, DM], F32, kind="ExternalInput").ap()
    mv_in = nc.dram_tensor("memory_values", [M, E], F32, kind="ExternalInput").ap()
    wk_in = nc.dram_tensor("Wk", [DM, E], F32, kind="ExternalInput").ap()
    bk_in = nc.dram_tensor("bk", [DM], F32, kind="ExternalInput").ap()
    wo_in = nc.dram_tensor("Wo", [E, E], F32, kind="ExternalInput").ap()
    bo_in = nc.dram_tensor("bo", [E], F32, kind="ExternalInput").ap()
    out_ext = nc.dram_tensor("out", [S, E], F32, kind="ExternalOutput").ap()

    import contextlib
    with tile.TileContext(nc) as tc:
        loop_cm = tc.For_i(0, repeat, 1) if repeat > 1 else contextlib.nullcontext()
        with loop_cm:
          with (
            tc.tile_pool(name="pp", bufs=1) as pp,
            tc.tile_pool(name="wp", bufs=2) as wp,
            tc.tile_pool(name="psp", bufs=2, space="PSUM") as psp,
            tc.tile_pool(name="dp", bufs=1, space="DRAM") as dp,
          ):
            # ---------------- persistent SBUF ----------------
            qwkT = pp.tile([128, NE, DM], F16, tag="qwkT")
            qwoT = pp.tile([128, NE, E], F16, tag="qwoT")
            mkT = pp.tile([128, ND, M], F16, tag="mkT")
            mv_sb = pp.tile([128, NM, E], F16, tag="mv_sb")
            bk_sb = pp.tile([128, ND], F32, tag="bk_sb")
            bo_sb = pp.tile([1, E], F32, tag="bo_sb")
            bo_sc = pp.tile([1, E], F16, tag="bo_sc")
            ones_col = pp.tile([128, 8], F16, tag="ones_col")
            ones_row = pp.tile([1, 128], F16, tag="ones_row")
            hmax_buf = pp.tile([128, NT * TS * NOH], F32, tag="hmax_buf")
            xmax_buf = pp.tile([128, NT], F32, tag="xmax_buf")
            xmax_red = pp.tile([128, 1], F32, tag="xmax_red")
            hmax_red = pp.tile([128, 1], F32, tag="hmax_red")
            pr_max = pp.tile([128, 1], F32, tag="pr_max")
            pr_add = pp.tile([128, 1], F32, tag="pr_add")

            sc = {}
            for nm in ("gmax_x", "s_x", "inv_sx", "ws_k", "thr_k", "nthr_k",
                       "ws_o", "thr_o", "nthr_o", "sck", "gmax_h", "s_h",
                       "inv_sh", "sco", "inv_sco", "red1", "red2"):
                sc[nm] = pp.tile([1, 8], F32, name=f"sc_{nm}", tag=f"sc_{nm}")
            bc = {}
            for nm in ("inv_sx", "thr_k", "nthr_k", "thr_o", "nthr_o", "sck",
                       "inv_sh", "sco"):
                bc[nm] = pp.tile([128, 1], F32, name=f"bc_{nm}", tag=f"bc_{nm}")

            neg_shift = pp.tile([128, 1], F32, tag="neg_shift")
            ident32 = pp.tile([128, 128], F32, tag="ident32")
            ident16 = pp.tile([128, 128], F16, tag="ident16")
            nc.vector.memset(ones_col[:], 1.0)
            nc.vector.memset(ones_row[:], 1.0)
            nc.vector.memset(neg_shift[:], -EXP_SHIFT)
            from concourse.masks import make_identity
            make_identity(nc, ident32[:])
            make_identity(nc, ident16[:])

            # ---------------- DRAM scratch ----------------
            h_sp = [dp.tile([T, E], F16, tag="h_spill", bufs=NT, name=f"hsp{i}")
                    for i in range(NT)]
            hT_sp = [dp.tile([E, T], F16, tag="hT_spill", bufs=NT, name=f"hTsp{i}")
                     for i in range(NT)]

            rg = [list(range(N_CORES))]

            def allreduce_max(src_scalar, dst_scalar):
                if not use_collectives:
                    nc.vector.tensor_copy(dst_scalar[0:1, 0:1], src_scalar[0:1, 0:1])
                    return
                ccin = dp.tile([1, 8], F32, tag="cc_in", bufs=2, name="ccin")
                ccout = dp.tile([1, 8], F32, addr_space="Shared", tag="cc_out",
                                bufs=2, name="ccout")
                nc.sync.dma_start(ccin[:], src_scalar[:])
                nc.gpsimd.collective_compute(
                    "AllReduce", mybir.AluOpType.max, replica_groups=rg,
                    ins=[ccin[:]], outs=[ccout[:]])
                nc.sync.dma_start(dst_scalar[:], ccout[:])

            def part_reduce_scalar(vec128, out_scalar, op):
                red = bass_isa.ReduceOp.max if op == "max" else bass_isa.ReduceOp.add
                dst = pr_max if op == "max" else pr_add
                nc.gpsimd.partition_all_reduce(dst[:], vec128[:], channels=128,
                                               reduce_op=red)
                nc.vector.tensor_copy(out_scalar[0:1, 0:1], dst[0:1, 0:1])

            # ============ PHASE A ============
            # x-max pass on the sync ring; weight loads on the SWDGE ring run
            # concurrently. Last two chunks are tiles 0/1 so their x_nat slots
            # are reused by phase B directly.
            chunk_order = list(range(2, NT)) + [0, 1]
            xa_ref = {}
            for c in chunk_order:
                xc = wp.tile([128, TS, E], F32, tag="x_nat", name="xc")
                nc.sync.dma_start(
                    xc[:], x_in[c * T:(c + 1) * T, :].rearrange("(n p) d -> p n d", p=128))
                nc.vector.tensor_reduce(
                    xmax_buf[:, c:c + 1], xc[:], axis=mybir.AxisListType.XY,
                    op=mybir.AluOpType.max, apply_absolute_value=True)
                xa_ref[c] = xc
            nc.vector.tensor_reduce(xmax_red[:], xmax_buf[:], axis=mybir.AxisListType.X,
                                    op=mybir.AluOpType.max)
            part_reduce_scalar(xmax_red, sc["red1"], "max")
            allreduce_max(sc["red1"], sc["gmax_x"])
            nc.vector.tensor_scalar(sc["s_x"][0:1, 0:1], sc["gmax_x"][0:1, 0:1],
                                    1.0 / 127.0, None, op0=mybir.AluOpType.mult)
            nc.vector.reciprocal(sc["inv_sx"][0:1, 0:1], sc["s_x"][0:1, 0:1])
            nc.gpsimd.partition_broadcast(bc["inv_sx"][:], sc["inv_sx"][0:1, 0:1])

            # ---- ternary weight prep (chunked; loads on SWDGE ring) ----
            def prep_ternary_weight(w_ap, rows, ws_t, thr_t, nthr_t, thr_b, nthr_b,
                                    dst, dstT):
                nblk = rows // 128
                nch = E // 512
                acc = wp.tile([128, nblk * nch], F32, tag="wacc", name="acc")
                wts = []
                for blk in range(nblk):
                    for ch in range(nch):
                        wt = wp.tile([128, 512], F32, tag="wld", bufs=3, name="wt")
                        nc.gpsimd.dma_start(
                            wt[:], w_ap[blk * 128:(blk + 1) * 128,
                                        ch * 512:(ch + 1) * 512])
                        nc.vector.tensor_reduce(
                            acc[:, blk * nch + ch:blk * nch + ch + 1], wt[:],
                            axis=mybir.AxisListType.X, op=mybir.AluOpType.add,
                            apply_absolute_value=True)
                accr = wp.tile([128, 1], F32, tag="waccr", name="accr")
                nc.vector.tensor_reduce(accr[:], acc[:], axis=mybir.AxisListType.X,
                                        op=mybir.AluOpType.add)
                part_reduce_scalar(accr, sc["red2"], "add")
                nc.vector.tensor_scalar(ws_t[0:1, 0:1], sc["red2"][0:1, 0:1],
                                        1.0 / (rows * E), None,
                                        op0=mybir.AluOpType.mult)
                nc.vector.tensor_scalar(thr_t[0:1, 0:1], ws_t[0:1, 0:1], 0.5, None,
                                        op0=mybir.AluOpType.mult)
                nc.vector.tensor_scalar(nthr_t[0:1, 0:1], thr_t[0:1, 0:1], -1.0, None,
                                        op0=mybir.AluOpType.mult)
                nc.gpsimd.partition_broadcast(thr_b[:], thr_t[0:1, 0:1])
                nc.gpsimd.partition_broadcast(nthr_b[:], nthr_t[0:1, 0:1])
                # quantize column-chunk-major so each dstT transpose unblocks early
                for ch in range(nch):
                    for blk in range(nblk):
                        wt = wp.tile([128, 512], F32, tag="wld", bufs=3, name="wt2")
                        nc.gpsimd.dma_start(
                            wt[:], w_ap[blk * 128:(blk + 1) * 128,
                                        ch * 512:(ch + 1) * 512])
                        ge = wp.tile([128, 512], F32, tag="f32b", bufs=3, name="ge")
                        nc.vector.tensor_scalar(ge[:], wt[:], thr_b[:, 0:1], None,
                                                op0=mybir.AluOpType.is_gt)
                        le = wp.tile([128, 512], F32, tag="f32b", bufs=3, name="le")
                        nc.vector.tensor_scalar(le[:], wt[:], nthr_b[:, 0:1], None,
                                                op0=mybir.AluOpType.is_lt)
                        q16 = wp.tile([128, 512], F16, tag="w16", name="q16")
                        nc.vector.tensor_tensor(q16[:], ge[:], le[:],
                                                op=mybir.AluOpType.subtract)
                        if dst is None:
                            # idle-PE transpose path (startup): q16 -> psum -> dstT
                            for j in range(4):
                                tps = psp.tile([128, 128], F16, tag="ps_d",
                                               name="wtp_ps")
                                nc.tensor.transpose(
                                    tps[:], q16[:, j * 128:(j + 1) * 128], ident16[:])
                                nc.scalar.activation(
                                    dstT[:, ch * 4 + j,
                                         blk * 128:(blk + 1) * 128], tps[:],
                                    mybir.ActivationFunctionType.Copy)
                        else:
                            nc.scalar.dma_start(
                                dst[blk * 128:(blk + 1) * 128,
                                    ch * 512:(ch + 1) * 512], q16[:])
                    if dst is not None:
                        for ib in range(ch * 4, (ch + 1) * 4):
                            nc.sync.dma_start_transpose(
                                dstT[:, ib, :], dst[0:rows, ib * 128:(ib + 1) * 128])

            prep_ternary_weight(wk_in, DM, sc["ws_k"], sc["thr_k"], sc["nthr_k"],
                                bc["thr_k"], bc["nthr_k"], None, qwkT)
            nc.vector.tensor_tensor(sc["sck"][0:1, 0:1], sc["ws_k"][0:1, 0:1],
                                    sc["s_x"][0:1, 0:1], op=mybir.AluOpType.mult)
            nc.gpsimd.partition_broadcast(bc["sck"][:], sc["sck"][0:1, 0:1])
            nc.gpsimd.dma_start(bk_sb[:], bk_in.rearrange("(b p) -> p b", p=128))

            # ---- memory bank: idle-PE transposes (no DRAM bounce) ----
            for mb in range(NM):
                for ch in range(DM // 512):
                    mkt = wp.tile([128, 512], F32, tag="wld", bufs=3, name="mkt")
                    nc.gpsimd.dma_start(mkt[:], mk_in[mb * 128:(mb + 1) * 128,
                                                      ch * 512:(ch + 1) * 512])
                    for j in range(4):
                        tps = psp.tile([128, 128], F32, tag="ps_d", name="mk_ps")
                        nc.tensor.transpose(
                            tps[:], mkt[:, j * 128:(j + 1) * 128], ident32[:])
                        nc.scalar.activation(
                            mkT[:, ch * 4 + j, mb * 128:(mb + 1) * 128], tps[:],
                            mybir.ActivationFunctionType.Copy)
            for mb in range(NM):
                for ch in range(E // 512):
                    mvt = wp.tile([128, 512], F32, tag="wld", bufs=3, name="mvt")
                    nc.gpsimd.dma_start(mvt[:], mv_in[mb * 128:(mb + 1) * 128,
                                                      ch * 512:(ch + 1) * 512])
                    nc.vector.tensor_copy(mv_sb[:, mb, ch * 512:(ch + 1) * 512], mvt[:])

            # ======================= PHASE B =======================
            def prep_b(it):
                """Load + quantize + bounce + transpose for tile `it`.
                Returns (x_nat, qxT)."""
                if it in (0, 1):
                    x_nat = xa_ref[it]
                else:
                    x_nat = wp.tile([128, TS, E], F32, tag="x_nat", name="x_nat")
                    nc.sync.dma_start(
                        x_nat[:], x_in[it * T:(it + 1) * T, :]
                        .rearrange("(n p) d -> p n d", p=128))
                qx_nat = wp.tile([128, TS, E], F16, tag="qx_nat", bufs=1, name="qx_nat")
                for n in range(TS):
                    for ch in range(E // 512):
                        off = ch * 512
                        t1 = wp.tile([128, 512], F32, tag="f32b", bufs=3, name="t1")
                        nc.vector.tensor_scalar(
                            t1[:], x_nat[:, n, off:off + 512], bc["inv_sx"][:, 0:1],
                            MAGIC, op0=mybir.AluOpType.mult, op1=mybir.AluOpType.add)
                        nc.vector.tensor_scalar(
                            qx_nat[:, n, off:off + 512], t1[:], MAGIC, None,
                            op0=mybir.AluOpType.subtract)
                qx_b = dp.tile([T, E], F16, tag="qx_b", bufs=2, name="qx_b")
                nc.scalar.dma_start(qx_b[:].rearrange("(n p) d -> p n d", p=128),
                                    qx_nat[:])
                qxT = wp.tile([128, NE, T], F16, tag="qxT", bufs=2, name="qxT")
                for ib in range(NE):
                    nc.sync.dma_start_transpose(qxT[:, ib, :],
                                                qx_b[:, ib * 128:(ib + 1) * 128])
                return x_nat, qxT

            def emit_hstage(it):
                hstage = wp.tile([128, NE, T], F16, tag="qxT", name="hstage")
                for ib in range(NE):
                    nc.sync.dma_start_transpose(hstage[:, ib, :],
                                                h_sp[it][:, ib * 128:(ib + 1) * 128])
                nc.gpsimd.dma_start(
                    hT_sp[it][:].rearrange("(a p) t -> p a t", p=128), hstage[:])

            nxt = prep_b(0)
            for it in range(NT):
                x_nat, qxT = nxt

                # qk^T: [Dm partitions, T free]
                qkT = wp.tile([128, ND, T], F16, tag="qkT", bufs=1, name="qkT")
                for db in range(ND):
                    for tch in range(NTC):
                        tf = slice(tch * 512, (tch + 1) * 512)
                        ps = psp.tile([128, 512], F32, tag="ps_a", name="qk_ps")
                        for ib in range(NE):
                            nc.tensor.matmul(
                                ps[:], qwkT[:, ib, db * 128:(db + 1) * 128],
                                qxT[:, ib, tf], start=(ib == 0), stop=(ib == NE - 1))
                        nc.scalar.activation(
                            qkT[:, db, tf], ps[:],
                            mybir.ActivationFunctionType.Identity,
                            bias=bk_sb[:, db:db + 1], scale=bc["sck"][:, 0:1])

                # software pipeline: emit next tile's prep early (after qk)
                if it + 1 < NT:
                    nxt = prep_b(it + 1)
                if it > 0:
                    emit_hstage(it - 1)

                # sims^T -> exp (shifted) -> fp16
                expT = wp.tile([128, NM, T], F16, tag="expT", bufs=1, name="expT")
                for mb in range(NM):
                    for tch in range(NTC):
                        tf = slice(tch * 512, (tch + 1) * 512)
                        ps = psp.tile([128, 512], F32, tag="ps_b", name="sims_ps")
                        for db in range(ND):
                            nc.tensor.matmul(
                                ps[:], mkT[:, db, mb * 128:(mb + 1) * 128],
                                qkT[:, db, tf], start=(db == 0), stop=(db == ND - 1))
                        nc.scalar.activation(
                            expT[:, mb, tf], ps[:],
                            mybir.ActivationFunctionType.Exp,
                            bias=neg_shift[:, 0:1], scale=1.0 / math.sqrt(DM))

                # retrieved (natural layout) + softmax denominator
                for tsub in range(TS):
                    tcol = slice(tsub * 128, (tsub + 1) * 128)
                    hidx = (it * TS + tsub) * NOH
                    dps = psp.tile([128, 8], F32, tag="ps_d", name="den_ps")
                    inv_t = wp.tile([128, 1], F32, tag="inv_t", bufs=4, name="inv_t")
                    for eh in range(NOH):
                        ef = slice(eh * 512, (eh + 1) * 512)
                        rps = psp.tile([128, 512], F32, tag="ps_r", name="r_ps")
                        for mb in range(NM):
                            nc.tensor.matmul(rps[:], expT[:, mb, tcol],
                                             mv_sb[:, mb, ef],
                                             start=(mb == 0), stop=(mb == NM - 1))
                            if eh == 0:
                                nc.tensor.matmul(dps[:, 0:1], expT[:, mb, tcol],
                                                 ones_col[:, 0:1],
                                                 start=(mb == 0), stop=(mb == NM - 1))
                        if eh == 0:
                            nc.vector.reciprocal(inv_t[:], dps[:, 0:1])
                        hch = wp.tile([128, 512], F16, tag="h16", bufs=3, name="hch")
                        nc.vector.scalar_tensor_tensor(
                            hch[:], rps[:], inv_t[:, 0:1], x_nat[:, tsub, ef],
                            op0=mybir.AluOpType.mult, op1=mybir.AluOpType.add)
                        nc.vector.tensor_reduce(
                            hmax_buf[:, hidx + eh:hidx + eh + 1], hch[:],
                            axis=mybir.AxisListType.X, op=mybir.AluOpType.max,
                            apply_absolute_value=True)
                        nc.scalar.dma_start(
                            h_sp[it][:].rearrange("(n p) d -> p n d", p=128)
                            [:, tsub, ef], hch[:])

                # Wo prep rides under phase B's PE work (needed only in phase C)
                if it == 0:
                    wo_b = dp.tile([E, E], F16, tag="w_b", bufs=2, name="wo_b")
                    prep_ternary_weight(wo_in, E, sc["ws_o"], sc["thr_o"],
                                        sc["nthr_o"], bc["thr_o"], bc["nthr_o"],
                                        wo_b, qwoT)
                    nc.gpsimd.dma_start(bo_sb[0:1, :],
                                        bo_in.rearrange("(a e) -> a e", a=1))

            emit_hstage(NT - 1)

            # prefetch first two hT tiles (independent of s_h / the allreduce)
            hT = {}

            def transp_c(it):
                t = wp.tile([128, NE, T], F16, tag="qxT", bufs=2, name="hT16")
                nc.sync.dma_start(
                    t[:], hT_sp[it][:].rearrange("(a p) t -> p a t", p=128))
                hT[it] = t

            transp_c(0)
            transp_c(1)

            # ---- global max|h| -> s_h, output scales ----
            nc.vector.tensor_reduce(hmax_red[:], hmax_buf[:], axis=mybir.AxisListType.X,
                                    op=mybir.AluOpType.max)
            part_reduce_scalar(hmax_red, sc["red1"], "max")
            allreduce_max(sc["red1"], sc["gmax_h"])
            nc.vector.tensor_scalar(sc["s_h"][0:1, 0:1], sc["gmax_h"][0:1, 0:1],
                                    1.0 / 127.0, None, op0=mybir.AluOpType.mult)
            nc.vector.reciprocal(sc["inv_sh"][0:1, 0:1], sc["s_h"][0:1, 0:1])
            nc.gpsimd.partition_broadcast(bc["inv_sh"][:], sc["inv_sh"][0:1, 0:1])
            nc.vector.tensor_tensor(sc["sco"][0:1, 0:1], sc["ws_o"][0:1, 0:1],
                                    sc["s_h"][0:1, 0:1], op=mybir.AluOpType.mult)
            nc.gpsimd.partition_broadcast(bc["sco"][:], sc["sco"][0:1, 0:1])
            nc.vector.reciprocal(sc["inv_sco"][0:1, 0:1], sc["sco"][0:1, 0:1])
            nc.vector.tensor_scalar(bo_sc[0:1, :], bo_sb[0:1, :],
                                    sc["inv_sco"][0:1, 0:1], None,
                                    op0=mybir.AluOpType.mult)

            # ======================= PHASE C =======================
            def quant_c(it):
                # two big flat ops (free dim NE*T) to amortize DVE DRAIN
                t = hT[it]
                t1 = wp.tile([128, TS, E], F32, tag="x_nat", name="t1c")
                t1f = t1.rearrange("p a b -> p (a b)")
                tf = t.rearrange("p a b -> p (a b)")
                nc.vector.tensor_scalar(
                    t1f[:, 0:NE * T], tf[:], bc["inv_sh"][:, 0:1],
                    MAGIC, op0=mybir.AluOpType.mult, op1=mybir.AluOpType.add)
                nc.vector.tensor_scalar(
                    tf[:], t1f[:, 0:NE * T], MAGIC, None,
                    op0=mybir.AluOpType.subtract)
                return t

            nxt_c = quant_c(0)
            for it in range(NT):
                qhT = nxt_c
                del hT[it]
                if it + 1 < NT:
                    nxt_c = quant_c(it + 1)
                for tsub in range(TS):
                    tcol = slice(tsub * 128, (tsub + 1) * 128)
                    for oh in range(NOH):
                        of = slice(oh * 512, (oh + 1) * 512)
                        ops = psp.tile([128, 512], F32,
                                       tag=("ps_a" if oh % 2 == 0 else "ps_b"),
                                       name="o_ps")
                        for ib in range(NE):
                            nc.tensor.matmul(ops[:], qhT[:, ib, tcol],
                                             qwoT[:, ib, of],
                                             start=(ib == 0),
                                             stop=(not with_bias and ib == NE - 1))
                        if with_bias:
                            nc.tensor.matmul(ops[:], ones_row[0:1, :],
                                             bo_sc[0:1, of],
                                             start=False, stop=True)
                        osb = wp.tile([128, 512], F32, tag="h16", bufs=3, name="osb")
                        nc.scalar.activation(
                            osb[:], ops[:], mybir.ActivationFunctionType.Copy,
                            bias=0.0, scale=bc["sco"][:, 0:1])
                        nc.gpsimd.dma_start(
                            out_ext[it * T:(it + 1) * T, :]
                            .rearrange("(n p) d -> p n d", p=128)
                            [:, tsub, of], osb[:])
                if it + 2 < NT:
                    transp_c(it + 2)

    nc.compile()
    return nc


# ----------------------------------------------------------------------------
_CACHE = {}


def _get_nc(key="full", **kw):
    if key not in _CACHE:
        _CACHE[key] = build_nc(**kw)
    return _CACHE[key]


def _make_in_maps(x, memory_keys, memory_values, Wk, bk, Wo, bo):
    x = np.ascontiguousarray(x, dtype=np.float32)
    shared = {
        "memory_keys": np.ascontiguousarray(memory_keys, dtype=np.float32),
        "memory_values": np.ascontiguousarray(memory_values, dtype=np.float32),
        "Wk": np.ascontiguousarray(Wk, dtype=np.float32),
        "bk": np.ascontiguousarray(bk, dtype=np.float32),
        "Wo": np.ascontiguousarray(Wo, dtype=np.float32),
        "bo": np.ascontiguousarray(bo, dtype=np.float32),
    }
    return [dict(shared, x=x[i]) for i in range(x.shape[0])]


def kernel(x, memory_keys, memory_values, Wk, bk, Wv=None, bv=None, Wo=None, bo=None):
    wb = bool(np.any(np.asarray(bo)))
    nc = _get_nc(("full", wb), with_bias=wb)
    in_maps = _make_in_maps(x, memory_keys, memory_values, Wk, bk, Wo, bo)
    res = run_bass_kernel_spmd(nc, in_maps, core_ids=list(range(N_CORES)))
    out = np.stack([res.results[i]["out"] for i in range(N_CORES)], axis=0)
    return out.astype(np.float32)


# ------------------------- benchmarking helper ------------------------------
def bench(inputs, iters=5, nc=None):
    """Time on-device execution with device-resident inputs."""
    import jax
    from jax.sharding import Mesh, PartitionSpec, NamedSharding
    from jax.experimental.shard_map import shard_map
    from concourse import bass2jax as b2j

    if nc is None:
        wb = bool(np.any(np.asarray(inputs["bo"])))
        nc = _get_nc(("full", wb), with_bias=wb)
    in_maps = _make_in_maps(inputs["x"], inputs["memory_keys"],
                            inputs["memory_values"], inputs["Wk"], inputs["bk"],
                            inputs["Wo"], inputs["bo"])
    b2j.install_neuronx_cc_hook()

    partition_name = nc.partition_id_tensor.name if nc.partition_id_tensor else None
    in_names, out_names, out_avals, zero_outs = [], [], [], []
    for alloc in nc.m.functions[0].allocations:
        if not isinstance(alloc, mybir.MemoryLocationSet):
            continue
        name = alloc.memorylocations[0].name
        if alloc.kind == "ExternalInput":
            if name != partition_name:
                in_names.append(name)
        elif alloc.kind == "ExternalOutput":
            out_names.append(name)
            shape = tuple(alloc.tensor_shape)
            dtype = mybir.dt.np(alloc.dtype)
            out_avals.append(jax.core.ShapedArray(shape, dtype))
            zero_outs.append(np.zeros(shape, dtype))
    n_params = len(in_names)
    n_outs = len(out_avals)
    in_names = in_names + out_names
    if partition_name is not None:
        in_names.append(partition_name)

    def _body(*args):
        operands = list(args)
        if partition_name is not None:
            operands.append(b2j.partition_id_tensor())
        outs = b2j._bass_exec_p.bind(
            *operands, out_avals=tuple(out_avals), in_names=tuple(in_names),
            out_names=tuple(out_names), lowering_input_output_aliases=(),
            sim_require_finite=True, sim_require_nnan=True, nc=nc)
        return tuple(outs)

    n_cores = len(in_maps)
    devices = jax.devices()[:n_cores]
    mesh = Mesh(np.asarray(devices), ("core",))
    in_specs = (PartitionSpec("core"),) * (n_params + n_outs)
    out_specs = (PartitionSpec("core"),) * len(out_names)
    donate = tuple(range(n_params, n_params + n_outs))
    sharded = jax.jit(
        shard_map(_body, mesh=mesh, in_specs=in_specs, out_specs=out_specs,
                  check_rep=False),
        donate_argnums=donate, keep_unused=True)

    per_core = [[np.asarray(m[nm]) for nm in in_names[:n_params]] for m in in_maps]
    concat_in = [np.concatenate([per_core[c][i] for c in range(n_cores)], axis=0)
                 for i in range(n_params)]
    sh = NamedSharding(mesh, PartitionSpec("core"))
    dev_in = [jax.device_put(a, sh) for a in concat_in]
    for a in dev_in:
        a.block_until_ready()

    times = []
    out_arrs = None
    for i in range(iters + 1):
        dev_zeros = [jax.device_put(
            np.zeros((n_cores * z.shape[0], *z.shape[1:]), z.dtype), sh)
            for z in zero_outs]
        for a in dev_zeros:
            a.block_until_ready()
        t0 = time.perf_counter()
        out_arrs = sharded(*dev_in, *dev_zeros)
        for o in out_arrs:
            o.block_until_ready()
        t1 = time.perf_counter()
        if i > 0:
            times.append(t1 - t0)
    oi = out_names.index("out")
    oshape = out_avals[oi].shape
    out = np.asarray(out_arrs[oi]).reshape(n_cores, *oshape)
    return times, out
